# revision 1
# baseline (speedup 1.0000x reference)
"""GRACE contrastive loss kernel for Trainium2 (8 NeuronCores, SPMD).

Strategy (row-block data parallel):
  - Shard the N=8192 nodes across 8 cores (1024 rows each).
  - Each core projects its z1/z2 block through the 2-layer MLP (fp16 matmuls,
    fp32 accum), computes per-node 1/norms as exp(-0.5*ln(sum h^2)) on the
    activation engine (avoids slow 1-lane reciprocals; Ln/Exp batched so the
    ACT table switches only twice per view), quantizes the normalized
    embeddings to fp8e4, and AllGathers them (fp8, 4.2 MB/view) so every
    core holds full gathered n1/n2 [512, 8192] in SBUF.
  - Similarities run as fp8 DoubleRow matmuls (K=256 per step, 2 steps) in
    [128 x 2048] PSUM groups (double-buffered, 8 banks) with fused exp(2*s)
    + row-sum on the scalar engine (accum_out).  Steady state is jointly
    scalar/PE bound at ~2.2us per 2048-column group.
  - Only S11, S12, S22 are computed explicitly; S21's row sums (= column
    sums of exp(S12) over the full matrix) come from a DVE fp16 2x
    accumulation of the exp(S12) tiles into a [128, 8192] buffer, fp16
    ones-matmul partition reductions interleaved into the first four S22
    row-tiles (borrowing a sim-psum slot), and a ReduceScatter(add) that
    overlaps the back half of S22.
  - The positive diagonal s12_ii is computed exactly in fp32 from h1/h2.
  - Per-core scalar partial out; host sums partials / N.

Measured: 352.9 us HW exec (baseline 718.9 us), rel err 1.2e-05.
"""

import math
import sys

import numpy as np

sys.path.insert(0, "/opt/trn_rl_repo")

import concourse.bass as bass  # noqa: E402
import concourse.mybir as mybir  # noqa: E402
import concourse.tile as tile  # noqa: E402
from concourse import bacc  # noqa: E402
from concourse.bass_utils import run_bass_kernel_spmd  # noqa: E402

F32 = mybir.dt.float32
F32R = mybir.dt.float32r
F16 = mybir.dt.float16
F8 = mybir.dt.float8e4
AF = mybir.ActivationFunctionType
ALU = mybir.AluOpType
DR = mybir.MatmulPerfMode.DoubleRow

N_CORES = 8
N = 8192
D = 512            # feature dim (= H = P in the reference MLP)
NB = N // N_CORES  # 1024 rows per core
KT = D // 128      # 4 k-subtiles
MT = NB // 128     # 8 row tiles per core
NCHUNK = 512
GW = 2048          # sim column-group width (4 psum banks)
NG = N // GW       # 4 groups per row tile
TAU_INV = 2.0      # 1 / tau
E2 = float(np.exp(2.0, dtype=np.float64))  # exp(diag(refl_sim)/tau), diag == 1

TRACE = False
LAST_EXEC_NS = None
_CACHE = {}


def _build_program(sim_mode=False):
    nc = bacc.Bacc("TRN2", target_bir_lowering=False, debug=False,
                   num_devices=1 if sim_mode else N_CORES)

    # ---- I/O ----
    zt1 = nc.dram_tensor("zt1", [128, KT, NB], F16, kind="ExternalInput").ap()
    zt2 = nc.dram_tensor("zt2", [128, KT, NB], F16, kind="ExternalInput").ap()
    w1t = nc.dram_tensor("w1t", [128, KT, D], F16, kind="ExternalInput").ap()
    w2t = nc.dram_tensor("w2t", [128, KT, D], F16, kind="ExternalInput").ap()
    b1 = nc.dram_tensor("b1", [128, KT], F32, kind="ExternalInput").ap()
    b2 = nc.dram_tensor("b2", [128, KT], F32, kind="ExternalInput").ap()
    out = nc.dram_tensor("out", [1, 1], F32, kind="ExternalOutput").ap()

    rg = [list(range(N_CORES))]

    with tile.TileContext(nc) as tc:
        with tc.tile_pool(name="persist", bufs=1) as persist, \
             tc.tile_pool(name="dram", bufs=1, space="DRAM") as dram, \
             tc.tile_pool(name="stats", bufs=1) as stats:

            ones_sc = persist.tile([1, 128], F32)
            nc.vector.memset(ones_sc[:], 1.0)
            ones_cs = persist.tile([128, 1], F32)
            nc.vector.memset(ones_cs[:], 1.0)
            ones_col = persist.tile([128, 1], F32R)
            nc.vector.tensor_copy(ones_col[:], ones_cs[:])
            ones_row = persist.tile([1, 128], F32R)
            nc.vector.tensor_copy(ones_row[:], ones_sc[:])
            ones_16 = persist.tile([128, 1], F16)
            nc.vector.memset(ones_16[:], 1.0)

            # normalized fp8 local blocks [feature, node] (sims lhsT)
            n8 = [persist.tile([128, KT, NB], F8, name=f"n8_{v}")
                  for v in range(2)]
            # 1/norm per node [1, NB]
            rn_vec = [persist.tile([1, NB], F32R, name=f"rn{v}") for v in range(2)]
            # gathered normalized embeddings, full row [feature, all nodes]
            g_sb = [persist.tile([128, KT, N], F8, name=f"g{v}") for v in range(2)]
            # colsum accumulator for exp(S12)
            acc = persist.tile([128, N], F16, name="acc")
            # fp32 projections (for the exact pos diagonal)
            h_sb = [persist.tile([128, KT, NB], F32, name=f"h{v}")
                    for v in range(2)]

            cc_in = [dram.tile([D, NB], F8, name=f"cc_in{v}") for v in range(2)]
            cc_out = [dram.tile([N_CORES * D, NB], F8, name=f"cc_out{v}",
                                addr_space="Shared",
                                tag=("agbuf0" if v == 0 else "agbuf1"))
                      for v in range(2)]
            cs_in = dram.tile([N], F32, name="cs_in")
            cs_out = dram.tile([NB], F32, name="cs_out")
            pos_part = stats.tile([1, NB], F32, name="pos_part")

            # exp row-sum partials per matrix: [128, MT, NG]
            parts = [stats.tile([128, MT, NG], F32, name=f"parts{x}")
                     for x in range(3)]  # 0=S11, 1=S12, 2=S22
            rs = [stats.tile([128, MT], F32, name=f"rs{x}") for x in range(3)]
            rs21 = stats.tile([128, MT], F32, name="rs21")
            d1g = stats.tile([128, MT], F32, name="d1g")
            pos_sum = stats.tile([1, 1], F32)

            # ---------------- projection phase ----------------
            with tc.tile_pool(name="proj", bufs=1) as proj, \
                 tc.tile_pool(name="ptmp", bufs=2) as ptmp, \
                 tc.tile_pool(name="ppsum", bufs=4, space="PSUM") as ppsum, \
                 tc.tile_pool(name="spsum", bufs=2, space="PSUM") as spsum:

                zt_sb = proj.tile([128, KT, NB], F16, name="zt_sb")
                w1_sb = proj.tile([128, KT, D], F16)
                w2_sb = proj.tile([128, KT, D], F16)
                b1_sb = proj.tile([128, KT], F32)
                b2_sb = proj.tile([128, KT], F32)
                e_sb = proj.tile([128, KT, NB], F16)
                hsq = proj.tile([128, KT, NB], F32R)

                nc.sync.dma_start(zt_sb[:], zt1)
                nc.sync.dma_start(w1_sb[:], w1t)
                nc.sync.dma_start(w2_sb[:], w2t)
                nc.sync.dma_start(b1_sb[:], b1)
                nc.sync.dma_start(b2_sb[:], b2)

                for v in range(2):
                    if v == 1:
                        nc.sync.dma_start(zt_sb[:], zt2)
                    # ---- layer 1 + ELU ----
                    for pt in range(KT):
                        for ch in range(NB // NCHUNK):
                            ps = ppsum.tile([128, NCHUNK], F32, tag="ps_proj")
                            for kt in range(KT):
                                nc.tensor.matmul(
                                    ps[:],
                                    lhsT=w1_sb[:, kt, pt * 128:(pt + 1) * 128],
                                    rhs=zt_sb[:, kt,
                                              ch * NCHUNK:(ch + 1) * NCHUNK],
                                    start=(kt == 0), stop=(kt == KT - 1))
                            # elu(y) = max(y,0) + min(exp(y),1) - 1,  y = ps + b1
                            texp = ptmp.tile([128, NCHUNK], F16, tag="texp")
                            nc.scalar.activation(texp[:], ps[:], AF.Exp,
                                                 bias=b1_sb[:, pt:pt + 1],
                                                 scale=1.0)
                            tclip = ptmp.tile([128, NCHUNK], F16, tag="tclip")
                            nc.vector.tensor_scalar(tclip[:], texp[:], 1.0, -1.0,
                                                    ALU.min, ALU.add)
                            tmax = ptmp.tile([128, NCHUNK], F16, tag="tmax")
                            nc.scalar.activation(tmax[:], ps[:], AF.Relu,
                                                 bias=b1_sb[:, pt:pt + 1],
                                                 scale=1.0)
                            nc.vector.tensor_tensor(
                                e_sb[:, pt, ch * NCHUNK:(ch + 1) * NCHUNK],
                                tmax[:], tclip[:], ALU.add)
                    # ---- layer 2 (+ b2 on DVE), squares on DVE ----
                    for jt in range(KT):
                        for ch in range(NB // NCHUNK):
                            ps = ppsum.tile([128, NCHUNK], F32, tag="ps_proj")
                            for kt in range(KT):
                                nc.tensor.matmul(
                                    ps[:],
                                    lhsT=w2_sb[:, kt, jt * 128:(jt + 1) * 128],
                                    rhs=e_sb[:, kt, ch * NCHUNK:(ch + 1) * NCHUNK],
                                    start=(kt == 0), stop=(kt == KT - 1))
                            sl = (slice(None), jt,
                                  slice(ch * NCHUNK, (ch + 1) * NCHUNK))
                            nc.vector.tensor_scalar(h_sb[v][sl], ps[:],
                                                    b2_sb[:, jt:jt + 1], None,
                                                    ALU.add)
                            nc.scalar.activation(hsq[sl], h_sb[v][sl], AF.Square)
                    # ---- per-node 1/norm: rn = exp(-0.5*ln(ss)).  Batch the
                    # Ln's then the Exp's so the ACT table switches only twice
                    # per view instead of per chunk.
                    tlns = []
                    for ch in range(NB // NCHUNK):
                        csl = slice(ch * NCHUNK, (ch + 1) * NCHUNK)
                        pss = spsum.tile([1, NCHUNK], F32, tag="ps_small")
                        for jt in range(KT):
                            nc.tensor.matmul(
                                pss[:],
                                lhsT=ones_col[:],
                                rhs=hsq[:, jt, csl],
                                start=(jt == 0), stop=(jt == KT - 1))
                        tln = ptmp.tile([1, NCHUNK], F32, tag="tln")
                        nc.scalar.activation(tln[:], pss[:], AF.Ln)
                        tlns.append(tln)
                    for ch in range(NB // NCHUNK):
                        csl = slice(ch * NCHUNK, (ch + 1) * NCHUNK)
                        nc.scalar.activation(rn_vec[v][:, csl], tlns[ch][:],
                                             AF.Exp, scale=-0.5)
                    for ch in range(NB // NCHUNK):
                        csl = slice(ch * NCHUNK, (ch + 1) * NCHUNK)
                        # broadcast rn across partitions (K=1 ones-matmul)
                        pbc = spsum.tile([128, NCHUNK], F32, tag="ps_bc")
                        nc.tensor.matmul(
                            pbc[:], lhsT=ones_row[:],
                            rhs=rn_vec[v][:, csl],
                            start=True, stop=True)
                        for jt in range(KT):
                            nc.vector.tensor_tensor(
                                n8[v][:, jt, csl], h_sb[v][:, jt, csl],
                                pbc[:], ALU.mult)

                    # ship to DRAM + AllGather (fp8; overlaps with the other
                    # view / the sims).  Per-kt DMAs: the first slice ships
                    # while later kt's quant is still running.
                    for kt in range(KT):
                        nc.sync.dma_start(
                            cc_in[v][kt * 128:(kt + 1) * 128, :],
                            n8[v][:, kt, :])
                    if sim_mode:
                        nc.sync.dma_start(cc_out[v][0:D, :], cc_in[v][:])
                    else:
                        nc.gpsimd.collective_compute(
                            "AllGather", ALU.bypass, replica_groups=rg,
                            ins=[cc_in[v].opt()], outs=[cc_out[v].opt()])

                # ---- pos diagonal: s12_ii = rn1_i*rn2_i*sum_f h1[f,i]h2[f,i]
                hh = hsq  # reuse
                for jt in range(KT):
                    nc.vector.tensor_tensor(hh[:, jt, :], h_sb[0][:, jt, :],
                                            h_sb[1][:, jt, :], ALU.mult)
                for ch in range(NB // NCHUNK):
                    csl = slice(ch * NCHUNK, (ch + 1) * NCHUNK)
                    psp = spsum.tile([1, NCHUNK], F32, tag="ps_small")
                    for jt in range(KT):
                        nc.tensor.matmul(psp[:],
                                         lhsT=ones_col[:],
                                         rhs=hh[:, jt, csl],
                                         start=(jt == 0), stop=(jt == KT - 1))
                    nc.vector.tensor_tensor(pos_part[:, csl], psp[:],
                                            rn_vec[0][:, csl], ALU.mult)
                    nc.vector.tensor_tensor(pos_part[:, csl], pos_part[:, csl],
                                            rn_vec[1][:, csl], ALU.mult)
                nc.vector.tensor_reduce(pos_sum[:], pos_part[:],
                                        mybir.AxisListType.X, ALU.add)

            # ---------------- load gathered embeddings ----------------
            for v in range(2):
                for r in range(N_CORES):
                    nc.sync.dma_start(
                        g_sb[v][:, :, r * NB:(r + 1) * NB],
                        cc_out[v][r * D:(r + 1) * D, :]
                        .rearrange("(ko p) m -> p ko m", p=128))

            # ---------------- sims: S11 then S12 ----------------
            # fp8 DoubleRow, K=256 per step.  [128, 2048] psum groups,
            # exp(2s) + row sums on ACT; S12's exp tiles also accumulate
            # into `acc` (DVE) for the S21 row sums (colsums of exp(S12)).
            def sim_pass(x, vl, vr, pool, scr, do_acc):
                for mt in range(MT):
                    for g in range(NG):
                        pss = pool.tile([128, GW], F32, tag="ps_sim")
                        for kt2 in range(KT // 2):
                            for ch in range(GW // NCHUNK):
                                c0 = g * GW + ch * NCHUNK
                                nc.tensor.matmul(
                                    pss[:, ch * NCHUNK:(ch + 1) * NCHUNK],
                                    lhsT=n8[vl][:, 2 * kt2:2 * kt2 + 2,
                                                mt * 128:(mt + 1) * 128],
                                    rhs=g_sb[vr][:, 2 * kt2:2 * kt2 + 2,
                                                 c0:c0 + NCHUNK],
                                    start=(kt2 == 0), stop=(kt2 == KT // 2 - 1),
                                    perf_mode=DR)
                        es = scr.tile([128, GW], F16, tag="es")
                        nc.scalar.activation(
                            es[:], pss[:], AF.Exp, scale=TAU_INV,
                            accum_out=parts[x][:, mt, g:g + 1])
                        if do_acc:
                            asl = acc[:, g * GW:(g + 1) * GW]
                            if mt == 0:
                                nc.vector.tensor_copy(asl, es[:])
                            else:
                                nc.vector.tensor_tensor(asl, asl, es[:],
                                                        ALU.add)

            with tc.tile_pool(name="sim_psum", bufs=2, space="PSUM") as sp, \
                 tc.tile_pool(name="scr", bufs=5) as scr, \
                 tc.tile_pool(name="cs_sbp", bufs=4) as cs_sbp:
                sim_pass(0, 0, 0, sp, scr, False)   # S11
                sim_pass(1, 0, 1, sp, scr, True)    # S12 (+ colsum acc)

                # d1 = ln(rs11 + rs12 - e^2) completes during S22
                nc.vector.tensor_reduce(rs[0][:], parts[0][:],
                                        mybir.AxisListType.X, ALU.add)
                nc.vector.tensor_reduce(rs[1][:], parts[1][:],
                                        mybir.AxisListType.X, ALU.add)
                nc.vector.tensor_tensor(d1g[:], rs[0][:], rs[1][:], ALU.add)
                nc.vector.tensor_scalar_add(d1g[:], d1g[:], -E2)
                nc.scalar.activation(d1g[:], d1g[:], AF.Ln)

                # ---- S22, with the S21 colsum reduction interleaved.
                # After row-tiles 2..5 of S22, borrow one sim-psum ring slot
                # for 4 ones-matmul partition reductions of `acc`; DVE copies
                # them out and small DMAs stream them to cs_in.  The
                # ReduceScatter then overlaps the tail of S22.
                for mt in range(MT):
                    for g in range(NG):
                        pss = sp.tile([128, GW], F32, tag="ps_sim")
                        for kt2 in range(KT // 2):
                            for ch in range(GW // NCHUNK):
                                c0 = g * GW + ch * NCHUNK
                                nc.tensor.matmul(
                                    pss[:, ch * NCHUNK:(ch + 1) * NCHUNK],
                                    lhsT=n8[1][:, 2 * kt2:2 * kt2 + 2,
                                               mt * 128:(mt + 1) * 128],
                                    rhs=g_sb[1][:, 2 * kt2:2 * kt2 + 2,
                                                c0:c0 + NCHUNK],
                                    start=(kt2 == 0), stop=(kt2 == KT // 2 - 1),
                                    perf_mode=DR)
                        es = scr.tile([128, GW], F16, tag="es")
                        nc.scalar.activation(
                            es[:], pss[:], AF.Exp, scale=TAU_INV,
                            accum_out=parts[2][:, mt, g:g + 1])
                    if mt <= 3:
                        rnd = mt
                        pcol = sp.tile([128, GW], F32, tag="ps_sim")
                        for i in range(4):
                            c = rnd * 4 + i
                            nc.tensor.matmul(
                                pcol[0:1, i * NCHUNK:(i + 1) * NCHUNK],
                                lhsT=ones_16[:],
                                rhs=acc[:, c * NCHUNK:(c + 1) * NCHUNK],
                                start=True, stop=True)
                        for i in range(4):
                            c = rnd * 4 + i
                            cst = cs_sbp.tile([1, NCHUNK], F32, tag="cs")
                            nc.vector.tensor_copy(
                                cst[:], pcol[0:1, i * NCHUNK:(i + 1) * NCHUNK])
                            nc.sync.dma_start(
                                cs_in[c * NCHUNK:(c + 1) * NCHUNK], cst[:])
                    if mt == 3:
                        if sim_mode:
                            nc.sync.dma_start(cs_out[:], cs_in[0:NB])
                        else:
                            nc.gpsimd.collective_compute(
                                "ReduceScatter", ALU.add, replica_groups=rg,
                                ins=[cs_in.opt()], outs=[cs_out.opt()])
                        nc.sync.dma_start(
                            rs21[:],
                            cs_out.rearrange("(mt p) -> p mt", p=128))

            # ---------------- assemble the loss ----------------
            with tc.tile_pool(name="fin", bufs=1) as fsb, \
                 tc.tile_pool(name="fin_psum", bufs=1, space="PSUM") as fp:
                nc.vector.tensor_reduce(rs[2][:], parts[2][:],
                                        mybir.AxisListType.X, ALU.add)
                d2 = fsb.tile([128, MT], F32)
                nc.vector.tensor_tensor(d2[:], rs[2][:], rs21[:], ALU.add)
                nc.vector.tensor_scalar_add(d2[:], d2[:], -E2)
                nc.scalar.activation(d2[:], d2[:], AF.Ln)
                lsum = fsb.tile([128, MT], F32)
                nc.vector.tensor_tensor(lsum[:], d1g[:], d2[:], ALU.add)
                lrow = fsb.tile([128, 1], F32)
                nc.vector.tensor_reduce(lrow[:], lsum[:],
                                        mybir.AxisListType.X, ALU.add)
                pfin = fp.tile([1, 1], F32)
                nc.tensor.matmul(pfin[:], lhsT=ones_cs[:], rhs=lrow[:],
                                 start=True, stop=True)
                fin = fsb.tile([1, 1], F32)
                nc.vector.tensor_scalar_mul(fin[:], pfin[:], 0.5)
                p2 = fsb.tile([1, 1], F32)
                nc.vector.tensor_scalar_mul(p2[:], pos_sum[:], 2.0)
                nc.vector.tensor_tensor(fin[:], fin[:], p2[:], ALU.subtract)
                nc.sync.dma_start(out, fin[:])

    nc.compile()
    return nc


def _prep_inputs(z1, z2, fc1_w, fc1_b, fc2_w, fc2_b):
    """Host-side shard + layout prep. Returns in_maps for the 8 cores."""
    w1t = np.ascontiguousarray(fc1_w.T).reshape(KT, 128, D).transpose(1, 0, 2)
    w1t = np.ascontiguousarray(w1t, dtype=np.float16)
    w2t = np.ascontiguousarray(fc2_w.T).reshape(KT, 128, D).transpose(1, 0, 2)
    w2t = np.ascontiguousarray(w2t, dtype=np.float16)
    b1 = np.ascontiguousarray(fc1_b.reshape(KT, 128).T, dtype=np.float32)
    b2 = np.ascontiguousarray(fc2_b.reshape(KT, 128).T, dtype=np.float32)

    in_maps = []
    for c in range(N_CORES):
        blk1 = z1[c * NB:(c + 1) * NB].T            # [512, 1024]
        blk2 = z2[c * NB:(c + 1) * NB].T
        zt1 = np.ascontiguousarray(
            blk1.reshape(KT, 128, NB).transpose(1, 0, 2), dtype=np.float16)
        zt2 = np.ascontiguousarray(
            blk2.reshape(KT, 128, NB).transpose(1, 0, 2), dtype=np.float16)
        in_maps.append({"zt1": zt1, "zt2": zt2, "w1t": w1t, "w2t": w2t,
                        "b1": b1, "b2": b2})
    return in_maps


def kernel(z1, z2, fc1_w, fc1_b, fc2_w, fc2_b):
    global LAST_EXEC_NS
    z1 = np.asarray(z1, dtype=np.float32)
    z2 = np.asarray(z2, dtype=np.float32)
    fc1_w = np.asarray(fc1_w, dtype=np.float32)
    fc1_b = np.asarray(fc1_b, dtype=np.float32)
    fc2_w = np.asarray(fc2_w, dtype=np.float32)
    fc2_b = np.asarray(fc2_b, dtype=np.float32)

    if "nc" not in _CACHE:
        _CACHE["nc"] = _build_program()
    nc = _CACHE["nc"]

    in_maps = _prep_inputs(z1, z2, fc1_w, fc1_b, fc2_w, fc2_b)
    res = run_bass_kernel_spmd(nc, in_maps, core_ids=list(range(N_CORES)),
                               trace=TRACE)
    LAST_EXEC_NS = res.exec_time_ns
    total = math.fsum(float(r["out"][0, 0]) for r in res.results)
    return np.float32(total / N)



# revision 6
# speedup vs baseline: 1.2801x; 1.2801x over previous
"""GRACE contrastive loss kernel for Trainium2 (8 NeuronCores, SPMD).

Strategy (symmetric row-block data parallel, fp8 everywhere):
  - Shard the N=8192 nodes across 8 cores (1024 rows each).  Projection
    MLP runs in fp8 DoubleRow (weights + activations quantized, ELU's
    "-1" folded into b2 host-side), fp32 accumulation; per-node 1/norms
    via exp(-0.5*ln(sum h^2)); normalized embeddings quantized to fp8
    and AllGather'd per view (a tiny dummy AllGather issued at t=0
    pulls the collective entry barrier off the critical path).
  - S11/S22 are symmetric: each core computes only shifts d=0..4 of its
    block row (d = (col_block - core) mod 8); row sums of the computed
    exp-blocks cover d=0..4, the d=5..7 contributions arrive as column
    sums computed by neighbor cores (shift 8-d in {1,2,3}), routed via
    one fp32 ReduceScatter.  The d=4 block is computed redundantly by
    both cores of its pair (no colsum exchange) to keep the SPMD
    program uniform.  S12 is computed in full per row; its column sums
    (= S21 row sums) ride the same ReduceScatter.
  - The gathered embeddings are loaded into SBUF in *rotated* order
    (slot d holds global block (core+d) mod 8) using dynamic-offset
    DMAs driven by the partition id, so all matmul addressing is
    uniform across cores.  Shift-0 (diagonal) blocks use the local
    embeddings directly and run while the AllGathers are in flight.
  - Sim groups are fp8 DoubleRow matmuls (N=1024 moving operand) into
    [128 x 2048] (or 1024) PSUM groups with fused exp(2s) + row-sum
    (accum_out) on the scalar engine; exp tiles for colsum-contributing
    blocks accumulate into fp16 buffers (DVE), partition-reduced by
    ones-matmuls and scattered into the ReduceScatter input at
    dynamically-computed (core-relative) offsets.  All Ln's run at the
    very end so the ACT exp table is loaded only once per set.
  - Per-core scalar partial out; host sums partials / N.
"""

import math
import sys

import numpy as np

sys.path.insert(0, "/opt/trn_rl_repo")

import concourse.bass as bass  # noqa: E402
import concourse.mybir as mybir  # noqa: E402
import concourse.tile as tile  # noqa: E402
from concourse import bacc  # noqa: E402
from concourse.bass_utils import run_bass_kernel_spmd  # noqa: E402

F32 = mybir.dt.float32
F32R = mybir.dt.float32r
F16 = mybir.dt.float16
F8 = mybir.dt.float8e4
AF = mybir.ActivationFunctionType
ALU = mybir.AluOpType
DR = mybir.MatmulPerfMode.DoubleRow

N_CORES = 8
N = 8192
D = 512            # feature dim (= H = P in the reference MLP)
NB = N // N_CORES  # 1024 rows per core
KT = D // 128      # 4 k-subtiles
MT = NB // 128     # 8 row tiles per core
NCHUNK = 512       # projection matmul moving width
SIMW = 1024        # sim matmul moving width (max for fp8)
BLK = D * NB       # elements in one gathered fp8 block
TAU_INV = 2.0      # 1 / tau
E2 = float(np.exp(2.0, dtype=np.float64))

TRACE = False
LAST_EXEC_NS = None
_CACHE = {}


def _build_program(sim_mode=False):
    nc = bacc.Bacc("TRN2", target_bir_lowering=False, debug=False,
                   num_devices=N_CORES)

    # ---- I/O ----
    zt1 = nc.dram_tensor("zt1", [128, KT, NB], F8, kind="ExternalInput").ap()
    zt2 = nc.dram_tensor("zt2", [128, KT, NB], F8, kind="ExternalInput").ap()
    w1t = nc.dram_tensor("w1t", [128, KT, D], F8, kind="ExternalInput").ap()
    w2t = nc.dram_tensor("w2t", [128, KT, D], F8, kind="ExternalInput").ap()
    b1 = nc.dram_tensor("b1", [128, KT], F32, kind="ExternalInput").ap()
    b2p = nc.dram_tensor("b2p", [128, KT], F32, kind="ExternalInput").ap()
    out = nc.dram_tensor("out", [1, 1], F32, kind="ExternalOutput").ap()

    rg = [list(range(N_CORES))]

    with tile.TileContext(nc) as tc:
        with tc.tile_pool(name="persist", bufs=1) as persist, \
             tc.tile_pool(name="dram", bufs=1, space="DRAM") as dram, \
             tc.tile_pool(name="stats", bufs=1) as stats:

            ones_cs = persist.tile([128, 1], F32)
            nc.vector.memset(ones_cs[:], 1.0)
            ones_col = persist.tile([128, 1], F32R)
            nc.vector.tensor_copy(ones_col[:], ones_cs[:])
            ones_sc = persist.tile([1, 128], F32)
            nc.vector.memset(ones_sc[:], 1.0)
            ones_row = persist.tile([1, 128], F32R)
            nc.vector.tensor_copy(ones_row[:], ones_sc[:])
            ones_16 = persist.tile([128, 1], F16)
            nc.vector.memset(ones_16[:], 1.0)

            # normalized fp8 local blocks [feature, node] (sims lhsT + d0 rhs)
            n8 = [persist.tile([128, KT, NB], F8, name=f"n8_{v}")
                  for v in range(2)]
            rn_vec = [persist.tile([1, NB], F32R, name=f"rn{v}") for v in range(2)]
            # rotated gathered embeddings: slot di holds global block
            # (core + di + 1) mod 8
            g1 = persist.tile([128, KT, 4 * NB], F8, name="g1")
            g2 = persist.tile([128, KT, 7 * NB], F8, name="g2")
            # colsum accumulators (rotated slot order)
            acc11 = persist.tile([128, 3 * NB], F16, name="acc11")
            acc22 = persist.tile([128, 3 * NB], F16, name="acc22")
            acc12 = persist.tile([128, 8 * NB], F16, name="acc12")

            # DRAM buffers
            shr = {} if sim_mode else {"addr_space": "Shared"}
            db_in = dram.tile([1, 8], F32, name="db_in")
            db_out = dram.tile([1, 64], F32, name="db_out",
                               tag="dbbuf", **shr)
            cc_in = [dram.tile([D, NB], F8, name=f"cc_in{v}") for v in range(2)]
            ccf = [dram.tile([1, N_CORES * BLK], F8, name=f"cc_out{v}",
                             tag=("agbuf0" if v == 0 else "agbuf1"), **shr)
                   for v in range(2)]
            cs_in = dram.tile([1, N_CORES * 3 * NB], F32, name="cs_in")
            cs_out = dram.tile([3 * NB], F32, name="cs_out")

            pos_part = stats.tile([1, NB], F32, name="pos_part")
            # accum_out row-sum partials: [128, MT, slots]
            parts11 = stats.tile([128, MT, 4], F32, name="parts11")
            parts12 = stats.tile([128, MT, 5], F32, name="parts12")
            parts22 = stats.tile([128, MT, 4], F32, name="parts22")
            pos_sum = stats.tile([1, 1], F32)

            # ---- dummy collective: pull the entry barrier to t=0 ----
            zcs = persist.tile([1, 3 * NB], F32, name="zcs")
            nc.vector.memset(zcs[:], 0.0)
            nc.sync.dma_start(db_in[:], zcs[:, 0:8])
            if not sim_mode:
                nc.gpsimd.collective_compute(
                    "AllGather", ALU.bypass, replica_groups=rg,
                    ins=[db_in.opt()], outs=[db_out.opt()])
            # zero-init the ReduceScatter input (S11/S22 sections of
            # slots this core does not write must contribute zero)
            for j in range(N_CORES):
                nc.sync.dma_start(cs_in[:, j * 3 * NB:(j + 1) * 3 * NB], zcs[:])

            # ---------------- projection phase ----------------
            with tc.tile_pool(name="proj", bufs=1) as proj, \
                 tc.tile_pool(name="ptmp", bufs=2) as ptmp, \
                 tc.tile_pool(name="ppsum", bufs=4, space="PSUM") as ppsum, \
                 tc.tile_pool(name="spsum", bufs=2, space="PSUM") as spsum:

                zt_sb = [proj.tile([128, KT, NB], F8, name=f"zt_sb{v}")
                         for v in range(2)]
                w1_sb = proj.tile([128, KT, D], F8)
                w2_sb = proj.tile([128, KT, D], F8)
                b1_sb = proj.tile([128, KT], F32)
                b2_sb = proj.tile([128, KT], F32)
                u_sb = proj.tile([128, KT, NB], F8)   # ELU out + 1
                h_sb = [proj.tile([128, KT, NB], F32, name=f"h{v}")
                        for v in range(2)]
                hsq = proj.tile([128, KT, NB], F32R)

                nc.sync.dma_start(w1_sb[:], w1t)
                nc.sync.dma_start(b1_sb[:], b1)
                nc.sync.dma_start(zt_sb[0][:], zt1)
                nc.sync.dma_start(w2_sb[:], w2t)
                nc.sync.dma_start(b2_sb[:], b2p)
                nc.sync.dma_start(zt_sb[1][:], zt2)

                for v in range(2):
                    # ---- layer 1 + ELU (u = elu(y) + 1 >= 0) ----
                    for pt in range(KT):
                        for ch in range(NB // NCHUNK):
                            csl = slice(ch * NCHUNK, (ch + 1) * NCHUNK)
                            ps = ppsum.tile([128, NCHUNK], F32, tag="ps_proj")
                            for k2 in range(KT // 2):
                                nc.tensor.matmul(
                                    ps[:],
                                    lhsT=w1_sb[:, 2 * k2:2 * k2 + 2,
                                               pt * 128:(pt + 1) * 128],
                                    rhs=zt_sb[v][:, 2 * k2:2 * k2 + 2, csl],
                                    start=(k2 == 0), stop=(k2 == KT // 2 - 1),
                                    perf_mode=DR)
                            texp = ptmp.tile([128, NCHUNK], F16, tag="texp")
                            nc.scalar.activation(texp[:], ps[:], AF.Exp,
                                                 bias=b1_sb[:, pt:pt + 1],
                                                 scale=1.0)
                            tmax = ptmp.tile([128, NCHUNK], F16, tag="tmax")
                            nc.scalar.activation(tmax[:], ps[:], AF.Relu,
                                                 bias=b1_sb[:, pt:pt + 1],
                                                 scale=1.0)
                            # u = min(exp(y),1) + relu(y)
                            nc.vector.scalar_tensor_tensor(
                                u_sb[:, pt, csl], texp[:], 1.0, tmax[:],
                                ALU.min, ALU.add)
                    # ---- layer 2 (+ folded b2) + squares ----
                    for jt in range(KT):
                        for ch in range(NB // NCHUNK):
                            csl = slice(ch * NCHUNK, (ch + 1) * NCHUNK)
                            ps = ppsum.tile([128, NCHUNK], F32, tag="ps_proj")
                            for k2 in range(KT // 2):
                                nc.tensor.matmul(
                                    ps[:],
                                    lhsT=w2_sb[:, 2 * k2:2 * k2 + 2,
                                               jt * 128:(jt + 1) * 128],
                                    rhs=u_sb[:, 2 * k2:2 * k2 + 2, csl],
                                    start=(k2 == 0), stop=(k2 == KT // 2 - 1),
                                    perf_mode=DR)
                            sl = (slice(None), jt, csl)
                            nc.vector.tensor_scalar(h_sb[v][sl], ps[:],
                                                    b2_sb[:, jt:jt + 1], None,
                                                    ALU.add)
                            nc.scalar.activation(hsq[sl], h_sb[v][sl], AF.Square)
                    # ---- 1/norm: rn = exp(-0.5*ln(ss)); Ln's batched ----
                    tlns = []
                    for ch in range(NB // NCHUNK):
                        csl = slice(ch * NCHUNK, (ch + 1) * NCHUNK)
                        pss = spsum.tile([1, NCHUNK], F32, tag="ps_small")
                        for jt in range(KT):
                            nc.tensor.matmul(
                                pss[:],
                                lhsT=ones_col[:],
                                rhs=hsq[:, jt, csl],
                                start=(jt == 0), stop=(jt == KT - 1))
                        tln = ptmp.tile([1, NCHUNK], F32, tag="tln")
                        nc.scalar.activation(tln[:], pss[:], AF.Ln)
                        tlns.append(tln)
                    for ch in range(NB // NCHUNK):
                        csl = slice(ch * NCHUNK, (ch + 1) * NCHUNK)
                        nc.scalar.activation(rn_vec[v][:, csl], tlns[ch][:],
                                             AF.Exp, scale=-0.5)
                    # ---- normalize + quantize to fp8 ----
                    for ch in range(NB // NCHUNK):
                        csl = slice(ch * NCHUNK, (ch + 1) * NCHUNK)
                        pbc = spsum.tile([128, NCHUNK], F32, tag="ps_bc")
                        nc.tensor.matmul(
                            pbc[:], lhsT=ones_row[:],
                            rhs=rn_vec[v][:, csl],
                            start=True, stop=True)
                        for jt in range(KT):
                            nc.vector.tensor_tensor(
                                n8[v][:, jt, csl], h_sb[v][:, jt, csl],
                                pbc[:], ALU.mult)
                    # ---- ship + AllGather ----
                    for kt in range(KT):
                        nc.sync.dma_start(
                            cc_in[v][kt * 128:(kt + 1) * 128, :],
                            n8[v][:, kt, :])
                    if sim_mode:
                        for r in range(N_CORES):
                            nc.sync.dma_start(
                                ccf[v][:, r * BLK:(r + 1) * BLK],
                                cc_in[v].rearrange("p m -> (p m)"))
                    else:
                        nc.gpsimd.collective_compute(
                            "AllGather", ALU.bypass, replica_groups=rg,
                            ins=[cc_in[v].opt()], outs=[ccf[v].opt()])

                # ---- pos diagonal: s12_ii = rn1_i*rn2_i*sum_f h1[f,i]h2[f,i]
                hh = hsq  # reuse
                for jt in range(KT):
                    nc.vector.tensor_tensor(hh[:, jt, :], h_sb[0][:, jt, :],
                                            h_sb[1][:, jt, :], ALU.mult)
                for ch in range(NB // NCHUNK):
                    csl = slice(ch * NCHUNK, (ch + 1) * NCHUNK)
                    psp = spsum.tile([1, NCHUNK], F32, tag="ps_small")
                    for jt in range(KT):
                        nc.tensor.matmul(psp[:],
                                         lhsT=ones_col[:],
                                         rhs=hh[:, jt, csl],
                                         start=(jt == 0), stop=(jt == KT - 1))
                    nc.vector.tensor_tensor(pos_part[:, csl], psp[:],
                                            rn_vec[0][:, csl], ALU.mult)
                    nc.vector.tensor_tensor(pos_part[:, csl], pos_part[:, csl],
                                            rn_vec[1][:, csl], ALU.mult)
                nc.vector.tensor_reduce(pos_sum[:], pos_part[:],
                                        mybir.AxisListType.X, ALU.add)

            # ---------------- rotated gathered loads ----------------
            pid = nc.sync.partition_id()
            for d in range(1, 5):
                off = ((pid + d) % N_CORES) * BLK
                nc.sync.dma_start(
                    g1[:, :, (d - 1) * NB:d * NB],
                    ccf[0][:, bass.ds(off, BLK)]
                    .rearrange("o (ko p m) -> (o p) ko m", p=128, ko=KT))
            for d in range(1, 8):
                off = ((pid + d) % N_CORES) * BLK
                nc.sync.dma_start(
                    g2[:, :, (d - 1) * NB:d * NB],
                    ccf[1][:, bass.ds(off, BLK)]
                    .rearrange("o (ko p m) -> (o p) ko m", p=128, ko=KT))

            # ---------------- similarity passes ----------------
            # chunks: list of (lhs_view, rhs_tile_or_None(local n8), rhs_off)
            def sim_group(sp, scr, mt, width, chunks, parts_ap, acc_list,
                          first_touch):
                pss = sp.tile([128, 2048], F32, tag="ps_sim")
                for ci, (lv, rtile, roff) in enumerate(chunks):
                    for k2 in range(KT // 2):
                        for sc in range(SIMW // NCHUNK):
                            c0 = roff + sc * NCHUNK
                            o0 = ci * SIMW + sc * NCHUNK
                            nc.tensor.matmul(
                                pss[:, o0:o0 + NCHUNK],
                                lhsT=n8[lv][:, 2 * k2:2 * k2 + 2,
                                            mt * 128:(mt + 1) * 128],
                                rhs=rtile[:, 2 * k2:2 * k2 + 2, c0:c0 + NCHUNK],
                                start=(k2 == 0), stop=(k2 == KT // 2 - 1),
                                perf_mode=DR)
                es = scr.tile([128, width], F16,
                              tag=("es2" if width == 2048 else "es1"))
                nc.scalar.activation(es[:], pss[:, 0:width], AF.Exp,
                                     scale=TAU_INV, accum_out=parts_ap)
                for (acc, aoff, eoff, w2) in acc_list:
                    asl = acc[:, aoff:aoff + w2]
                    esl = es[:, eoff:eoff + w2]
                    if first_touch:
                        nc.vector.tensor_copy(asl, esl)
                    else:
                        nc.vector.tensor_tensor(asl, asl, esl, ALU.add)

            # partition-reduce ncol 1024-wide columns of an fp16 acc buffer
            # into one borrowed psum slot, stage to SBUF, scatter to cs_in.
            def colsum_flush(sp, cs_sbp, acc, aoff0, ncol, sec, d0):
                pcol = sp.tile([128, 2048], F32, tag="ps_sim")
                for i in range(ncol):
                    for sc in range(NB // NCHUNK):
                        o0 = i * NB + sc * NCHUNK
                        nc.tensor.matmul(
                            pcol[0:1, o0:o0 + NCHUNK],
                            lhsT=ones_16[:],
                            rhs=acc[:, aoff0 + o0:aoff0 + o0 + NCHUNK],
                            start=True, stop=True)
                for i in range(ncol):
                    cst = cs_sbp.tile([1, NB], F32, tag="cs")
                    nc.vector.tensor_copy(cst[:], pcol[0:1, i * NB:(i + 1) * NB])
                    d = d0 + i
                    woff = ((pid + d) % N_CORES) * (3 * NB) + sec * NB
                    nc.sync.dma_start(cs_in[:, bass.ds(woff, NB)], cst[:])

            with tc.tile_pool(name="sim_psum", bufs=2, space="PSUM") as sp, \
                 tc.tile_pool(name="scr", bufs=4) as scr, \
                 tc.tile_pool(name="cs_sbp", bufs=4) as cs_sbp:

                # P1: shift-0 diagonal blocks (local, overlap AllGathers)
                for mt in range(MT):
                    ft = (mt == 0)
                    sim_group(sp, scr, mt, 1024, [(0, n8[0], 0)],
                              parts11[:, mt, 0:1], [], ft)
                    sim_group(sp, scr, mt, 1024, [(1, n8[1], 0)],
                              parts22[:, mt, 0:1], [], ft)
                    sim_group(sp, scr, mt, 1024, [(0, n8[1], 0)],
                              parts12[:, mt, 0:1], [(acc12, 0, 0, NB)], ft)

                # P2: S11 shifts 1..3 (needs g1)
                for mt in range(MT):
                    ft = (mt == 0)
                    sim_group(sp, scr, mt, 2048,
                              [(0, g1, 0), (0, g1, NB)],
                              parts11[:, mt, 1:2], [(acc11, 0, 0, 2 * NB)], ft)
                    sim_group(sp, scr, mt, 1024, [(0, g1, 2 * NB)],
                              parts11[:, mt, 2:3], [(acc11, 2 * NB, 0, NB)], ft)

                # P3: S12 shifts 1..7 (needs g2); acc11 colsums flushed here
                for mt in range(MT):
                    ft = (mt == 0)
                    sim_group(sp, scr, mt, 2048,
                              [(0, g2, 0), (0, g2, NB)],
                              parts12[:, mt, 1:2], [(acc12, NB, 0, 2 * NB)], ft)
                    sim_group(sp, scr, mt, 2048,
                              [(0, g2, 2 * NB), (0, g2, 3 * NB)],
                              parts12[:, mt, 2:3],
                              [(acc12, 3 * NB, 0, 2 * NB)], ft)
                    sim_group(sp, scr, mt, 2048,
                              [(0, g2, 4 * NB), (0, g2, 5 * NB)],
                              parts12[:, mt, 3:4],
                              [(acc12, 5 * NB, 0, 2 * NB)], ft)
                    sim_group(sp, scr, mt, 1024, [(0, g2, 6 * NB)],
                              parts12[:, mt, 4:5], [(acc12, 7 * NB, 0, NB)], ft)
                    if mt == 0:
                        colsum_flush(sp, cs_sbp, acc11, 0, 2, 1, 1)
                    elif mt == 1:
                        colsum_flush(sp, cs_sbp, acc11, 2 * NB, 1, 1, 3)

                # P4: S22 shifts 1..3; acc12 colsums flushed here
                for mt in range(MT):
                    ft = (mt == 0)
                    sim_group(sp, scr, mt, 2048,
                              [(1, g2, 0), (1, g2, NB)],
                              parts22[:, mt, 1:2], [(acc22, 0, 0, 2 * NB)], ft)
                    sim_group(sp, scr, mt, 1024, [(1, g2, 2 * NB)],
                              parts22[:, mt, 2:3], [(acc22, 2 * NB, 0, NB)], ft)
                    if mt < 4:
                        colsum_flush(sp, cs_sbp, acc12, mt * 2 * NB, 2, 0,
                                     mt * 2)

                # P5: redundant shift-4 blocks (no colsums) + acc22 flush + RS
                for mt in range(MT):
                    sim_group(sp, scr, mt, 1024, [(0, g1, 3 * NB)],
                              parts11[:, mt, 3:4], [], False)
                    sim_group(sp, scr, mt, 1024, [(1, g2, 3 * NB)],
                              parts22[:, mt, 3:4], [], False)
                    if mt == 0:
                        colsum_flush(sp, cs_sbp, acc22, 0, 2, 2, 1)
                    elif mt == 1:
                        colsum_flush(sp, cs_sbp, acc22, 2 * NB, 1, 2, 3)
                    elif mt == 2:
                        if sim_mode:
                            nc.sync.dma_start(cs_out[:],
                                              cs_in[:, 0:3 * NB]
                                              .rearrange("o m -> (o m)"))
                        else:
                            nc.gpsimd.collective_compute(
                                "ReduceScatter", ALU.add, replica_groups=rg,
                                ins=[cs_in.opt()], outs=[cs_out.opt()])

            # ---------------- assemble the loss ----------------
            with tc.tile_pool(name="fin", bufs=1) as fsb, \
                 tc.tile_pool(name="fin_psum", bufs=1, space="PSUM") as fp:
                rs11 = fsb.tile([128, MT], F32)
                rs12 = fsb.tile([128, MT], F32)
                rs22 = fsb.tile([128, MT], F32)
                nc.vector.tensor_reduce(rs11[:], parts11[:],
                                        mybir.AxisListType.X, ALU.add)
                nc.vector.tensor_reduce(rs12[:], parts12[:],
                                        mybir.AxisListType.X, ALU.add)
                nc.vector.tensor_reduce(rs22[:], parts22[:],
                                        mybir.AxisListType.X, ALU.add)
                r21 = fsb.tile([128, MT], F32)
                r11 = fsb.tile([128, MT], F32)
                r22 = fsb.tile([128, MT], F32)
                nc.sync.dma_start(
                    r21[:], cs_out[0:NB].rearrange("(mt p) -> p mt", p=128))
                nc.sync.dma_start(
                    r11[:], cs_out[NB:2 * NB].rearrange("(mt p) -> p mt", p=128))
                nc.sync.dma_start(
                    r22[:], cs_out[2 * NB:3 * NB]
                    .rearrange("(mt p) -> p mt", p=128))

                d1 = fsb.tile([128, MT], F32)
                nc.vector.tensor_tensor(d1[:], rs11[:], r11[:], ALU.add)
                nc.vector.tensor_tensor(d1[:], d1[:], rs12[:], ALU.add)
                nc.vector.tensor_scalar_add(d1[:], d1[:], -E2)
                d2 = fsb.tile([128, MT], F32)
                nc.vector.tensor_tensor(d2[:], rs22[:], r22[:], ALU.add)
                nc.vector.tensor_tensor(d2[:], d2[:], r21[:], ALU.add)
                nc.vector.tensor_scalar_add(d2[:], d2[:], -E2)
                nc.scalar.activation(d1[:], d1[:], AF.Ln)
                nc.scalar.activation(d2[:], d2[:], AF.Ln)
                lsum = fsb.tile([128, MT], F32)
                nc.vector.tensor_tensor(lsum[:], d1[:], d2[:], ALU.add)
                lrow = fsb.tile([128, 1], F32)
                nc.vector.tensor_reduce(lrow[:], lsum[:],
                                        mybir.AxisListType.X, ALU.add)
                pfin = fp.tile([1, 1], F32)
                nc.tensor.matmul(pfin[:], lhsT=ones_cs[:], rhs=lrow[:],
                                 start=True, stop=True)
                fin = fsb.tile([1, 1], F32)
                nc.vector.tensor_scalar_mul(fin[:], pfin[:], 0.5)
                p2 = fsb.tile([1, 1], F32)
                nc.vector.tensor_scalar_mul(p2[:], pos_sum[:], 2.0)
                nc.vector.tensor_tensor(fin[:], fin[:], p2[:], ALU.subtract)
                nc.sync.dma_start(out, fin[:])

    nc.compile()
    return nc


def _to_fp8(x):
    import ml_dtypes
    return np.asarray(x, dtype=ml_dtypes.float8_e4m3fn)


def _prep_inputs(z1, z2, fc1_w, fc1_b, fc2_w, fc2_b):
    """Host-side shard + layout prep. Returns in_maps for the 8 cores."""
    w1t = np.ascontiguousarray(fc1_w.T).reshape(KT, 128, D).transpose(1, 0, 2)
    w1t = _to_fp8(np.ascontiguousarray(w1t))
    w2t = np.ascontiguousarray(fc2_w.T).reshape(KT, 128, D).transpose(1, 0, 2)
    w2t = _to_fp8(np.ascontiguousarray(w2t))
    b1 = np.ascontiguousarray(fc1_b.reshape(KT, 128).T, dtype=np.float32)
    # ELU's "-1" folded: h = (elu(y)+1) @ w2.T + (b2 - w2.sum(axis=1))
    b2f = (fc2_b - fc2_w.sum(axis=1)).astype(np.float32)
    b2p = np.ascontiguousarray(b2f.reshape(KT, 128).T, dtype=np.float32)

    in_maps = []
    for c in range(N_CORES):
        blk1 = z1[c * NB:(c + 1) * NB].T            # [512, 1024]
        blk2 = z2[c * NB:(c + 1) * NB].T
        zt1 = _to_fp8(np.ascontiguousarray(
            blk1.reshape(KT, 128, NB).transpose(1, 0, 2)))
        zt2 = _to_fp8(np.ascontiguousarray(
            blk2.reshape(KT, 128, NB).transpose(1, 0, 2)))
        in_maps.append({"zt1": zt1, "zt2": zt2, "w1t": w1t, "w2t": w2t,
                        "b1": b1, "b2p": b2p})
    return in_maps


def kernel(z1, z2, fc1_w, fc1_b, fc2_w, fc2_b):
    global LAST_EXEC_NS
    z1 = np.asarray(z1, dtype=np.float32)
    z2 = np.asarray(z2, dtype=np.float32)
    fc1_w = np.asarray(fc1_w, dtype=np.float32)
    fc1_b = np.asarray(fc1_b, dtype=np.float32)
    fc2_w = np.asarray(fc2_w, dtype=np.float32)
    fc2_b = np.asarray(fc2_b, dtype=np.float32)

    if "nc" not in _CACHE:
        _CACHE["nc"] = _build_program()
    nc = _CACHE["nc"]

    in_maps = _prep_inputs(z1, z2, fc1_w, fc1_b, fc2_w, fc2_b)
    res = run_bass_kernel_spmd(nc, in_maps, core_ids=list(range(N_CORES)),
                               trace=TRACE)
    LAST_EXEC_NS = res.exec_time_ns
    total = math.fsum(float(r["out"][0, 0]) for r in res.results)
    return np.float32(total / N)
